# revision 60
# baseline (speedup 1.0000x reference)
"""Multi-head attention (B=4, S=2048, D=1024, H=16) on 8 trn2 NeuronCores.

Sharding: tensor-parallel over heads - 2 heads per core. Each core computes
qkv for its 128 channels (column-parallel), full attention for its 8
(batch, head) pairs, and a row-parallel slice of the output projection.
Host gathers the 8 partial projection outputs (bf16) and sums them in
float64 (+ b_proj).

All matmul operands are bf16 (PSUM accumulation stays f32). Matmul time
is (moving columns) * ~0.42 ns, so the layout minimizes streamed columns:

  - Scores use a ZERO-PADDED kT: for each 128-key tile and head h, a
    [128,128] stationary block holds kT in partition rows 64h..64h+63 and
    zeros elsewhere; the moving operand is the stacked qT (head0 channels
    in partitions 0-63, head1 in 64-127). The zero rows kill the other
    head's contribution, and the output's full 128-key width is used, so
    scores hit the output-bound floor of S^2/128 cycles per (b,h).
  - attn@V is FLIPPED: stationary = pT [128 keys, 128 queries] (a slice
    of the exp output), moving = v [128 keys, 65]. That streams 65
    columns per (key tile x query tile) instead of 128+ per key tile —
    LDWEIGHTS of the next stationary hides behind the moving stream.
    v carries a ones column, so each accumulator poT [128 q, 65] ends
    with the softmax denominator at column 64: normalization is a plain
    per-partition DVE reciprocal + tensor_scalar multiply (no GpSimd),
    then a PE transpose puts attn-out back into [ch, tok] layout for the
    row-parallel projection.

Scheduling: softmax exp makes the ScalarE the steady-state pacer (~1.04us
per [128,1024] chunk), so each (head, qblock) iteration runs scores one
group ahead of attn@V, with filler work (output projection of batch b-1,
one 512-token qkv part of batch b+1 per iteration, and for the last batch
its own ready projection blocks) emitted between sc(g+1) and av(g).
Batch 0's qkv streams inside its first attention iteration (group g's
scores only need key block g).

PSUM discipline (the tile scheduler only preserves data deps, and
pending-zero marks are 2KB-bank granular): every non-accumulating matmul
write uses start=True, and every accumulation chain owns a whole bank
(one poT tile per 128-query tile), so no chain's start can be reordered
against another chain's first write.
"""

import numpy as np
import ml_dtypes

import concourse.bass as bass
import concourse.mybir as mybir
import concourse.tile as tile
from concourse import bacc
from concourse.bass_utils import run_bass_kernel_spmd
from concourse.masks import make_identity

F32 = mybir.dt.float32
BF16 = mybir.dt.bfloat16

N_CORES = 8


def build_core_program(B=4, S=2048, D=1024, H=16, QB=256, TB=512, reps=1,
                       bufs_x=4, bufs_pT=3, bufs_s=2, bufs_o=2, bufs_wy=2,
                       KG=4, act_copy_frac=0):
    """One core's program (SPMD: every core runs this on its own shard).

    act_copy_frac: every act_copy_frac-th projection psum->sbuf copy goes
    to ScalarE instead of DVE (0 = all on DVE).
    """
    HD = D // H                 # 64
    HPC = H // N_CORES          # heads per core = 2
    CPC = HPC * HD              # channels per core = 128
    T = B * S                   # tokens = 8192
    SCALE = float(HD) ** -0.5
    KT = 128                    # ki tile
    NKT = S // KT               # ki tiles per batch = 16
    NTT = T // KT               # token tiles total = 64
    VW = HD + 1                 # v tile width per head incl. ones col = 65
    NQB = S // QB               # q blocks per batch = 8
    KD = D // 128               # contraction tiles for qkv = 8
    KH = KD // 2
    NG = NKT // KG              # score groups per q block = 4
    TPB = TB // 128             # 128-token tiles per qkv block = 4
    TBB = S // TB               # qkv token blocks per batch = 4
    NIT = HPC * NQB             # attention iterations per batch = 16

    nc = bacc.Bacc("TRN2", target_bir_lowering=False, debug=False,
                   num_devices=N_CORES)

    xT_d = nc.dram_tensor("xT", [D, T], BF16, kind="ExternalInput")
    wq_d = nc.dram_tensor("wq", [D, CPC], BF16, kind="ExternalInput")
    wk_d = nc.dram_tensor("wk", [D, CPC], BF16, kind="ExternalInput")
    wv_d = nc.dram_tensor("wv", [D, CPC], BF16, kind="ExternalInput")
    bq_d = nc.dram_tensor("bq", [CPC, 1], F32, kind="ExternalInput")
    bk_d = nc.dram_tensor("bk", [CPC, 1], F32, kind="ExternalInput")
    bv_d = nc.dram_tensor("bv", [CPC, 1], F32, kind="ExternalInput")
    wp_d = nc.dram_tensor("wp", [CPC, D], BF16, kind="ExternalInput")
    y_d = nc.dram_tensor("y", [T, D], BF16, kind="ExternalOutput")

    with tile.TileContext(nc) as tc:
        with tc.tile_pool(name="const", bufs=1) as const, \
             tc.tile_pool(name="persist", bufs=1) as persist, \
             tc.tile_pool(name="xin", bufs=bufs_x) as xin, \
             tc.tile_pool(name="vtmp", bufs=2) as vtmp, \
             tc.tile_pool(name="pT", bufs=bufs_pT) as p_pool, \
             tc.tile_pool(name="ao", bufs=2) as ao_pool, \
             tc.tile_pool(name="rcp", bufs=3) as rcp_pool, \
             tc.tile_pool(name="aot", bufs=3) as aot_pool, \
             tc.tile_pool(name="yout", bufs=3) as yout, \
             tc.tile_pool(name="s_ps", bufs=bufs_s, space="PSUM") as s_ps, \
             tc.tile_pool(name="o_ps", bufs=bufs_o, space="PSUM") as o_ps, \
             tc.tile_pool(name="wy_ps", bufs=bufs_wy, space="PSUM") as wy_ps:
            tr_ps = o_ps  # poT and tr tiles share the o_ps bank rotation
            xT_r = xT_d.ap().rearrange("(a p) t -> p a t", p=128)
            # per-batch persistent tiles: whole-tile WAR tracking would
            # otherwise serialize each rep's first qkv writes behind the
            # previous rep's LAST attention reads (and filler qkv writes
            # behind unrelated batches' reads), blocking cross-rep overlap
            qT_b = [persist.tile([128, S], BF16, name=f"qT_{b}")
                    for b in range(B)]             # stacked head dims
            kTp_b = [persist.tile([128, 2 * S], BF16, name=f"kTp_{b}")
                     for b in range(B)]            # zero-padded per head
            v_b = [persist.tile([128, NKT * HPC * VW], BF16, name=f"v_{b}")
                   for b in range(B)]
            # padded-kT views: [128, key-tile, head, 128]
            kTp_v = [t[:].rearrange("p (t h c) -> p t h c", h=HPC, c=KT)
                     for t in kTp_b]

            ident = const.tile([128, 128], BF16)
            ones_bf = const.tile([128, 2], BF16)
            wq_s = const.tile([128, KD, 128], BF16)
            wk_s = const.tile([128, KD, 128], BF16)
            wv_s = const.tile([128, KD, 128], BF16)
            wp_s = const.tile([128, D], BF16)
            bq_s = const.tile([CPC, 1], F32)
            bk_s = const.tile([CPC, 1], F32)
            bv_s = const.tile([CPC, 1], F32)

            def setup():
                # k's weights first: block 0's k projection gates the
                # first score matmul
                for w_d, w_s in ((wk_d, wk_s), (wq_d, wq_s), (wv_d, wv_s)):
                    nc.sync.dma_start(
                        w_s[:], w_d.ap().rearrange("(a p) m -> p a m", p=128))
                nc.sync.dma_start(bq_s[:], bq_d.ap()[:, :])
                nc.sync.dma_start(bk_s[:], bk_d.ap()[:, :])
                nc.sync.dma_start(bv_s[:], bv_d.ap()[:, :])
                make_identity(nc, ident[:])
                nc.vector.memset(ones_bf[:], 1.0)
                for bb in range(B):
                    nc.vector.memset(kTp_b[bb][:], 0.0)
                    ones_cols = v_b[bb][:].rearrange(
                        "p (t h w) -> p t h w", t=NKT, h=HPC)[:, :, :, HD:VW]
                    nc.vector.tensor_copy(
                        ones_cols,
                        ones_bf[:, 0:1].to_broadcast([128, NKT, HPC, 1]))

            def load_x(rep, tb, split=False):
                xt = xin.tile([128, KD, TB], BF16, name=f"xt_{rep}_{tb}",
                              tag="xt")
                if split:
                    # per-2-kd DMAs so the first qkv matmul (kd=0) can
                    # start as soon as its slice lands
                    for k0 in range(0, KD, 2):
                        nc.sync.dma_start(
                            xt[:, k0:k0 + 2, :],
                            xT_r[:, k0:k0 + 2, tb * TB:(tb + 1) * TB])
                else:
                    nc.sync.dma_start(xt[:], xT_r[:, :, tb * TB:(tb + 1) * TB])
                return xt

            def emit_qkv_tb(rep, tb, xt, parts=("q", "k", "v")):
                """qkv projections + padded-kT / transposed-v writes for one
                512-token block."""
                bb, ltb = divmod(tb, TBB)   # batch, block-within-batch
                lsl = slice(ltb * TB, (ltb + 1) * TB)
                sel = {"q": (wq_s, bq_s), "k": (wk_s, bk_s),
                       "v": (wv_s, bv_s)}
                for which in parts:
                    w_s, b_s = sel[which]
                    ps = wy_ps.tile([128, TB], F32,
                                    name=f"ps_{which}{rep}_{tb}", tag="wy")
                    for kd in range(KD):
                        nc.tensor.matmul(ps[:], w_s[:, kd, :], xt[:, kd, :],
                                         start=(kd == 0), stop=(kd == KD - 1))
                    if which == "q":
                        nc.vector.tensor_scalar_add(qT_b[bb][:, lsl], ps[:],
                                                    b_s[:])
                    elif which == "k":
                        ps_r = ps[:].rearrange("p (t c) -> p t c", c=KT)
                        for h in range(HPC):
                            hs = slice(h * HD, (h + 1) * HD)
                            dst = kTp_v[bb][hs, ltb * TPB:(ltb + 1) * TPB,
                                           h, :]
                            nc.vector.tensor_scalar_add(
                                dst, ps_r[hs, :, :], b_s[hs, :])
                    else:
                        vt = vtmp.tile([128, TB], BF16, name=f"vt{rep}_{tb}",
                                       tag="vt")
                        nc.vector.tensor_scalar_add(vt[:], ps[:], b_s[:])
                        for j in range(TPB):
                            ti = ltb * TPB + j
                            pt = wy_ps.tile([128, 128], BF16,
                                            name=f"pt{rep}_{tb}_{j}",
                                            tag="wy")
                            nc.tensor.transpose(
                                pt[:], vt[:, j * 128:(j + 1) * 128], ident[:])
                            base = ti * HPC * VW
                            dst = v_b[bb][:, base:base + HPC * VW].rearrange(
                                "p (h w) -> p h w", h=HPC)[:, :, 0:HD]
                            src = pt[:].rearrange("p (h w) -> p h w", h=HPC)
                            nc.vector.tensor_copy(dst, src)

            def emit_proj_tt(rep, b, ao, tt):
                """output projection for one 128-token block of batch b."""
                lt = ao[:, tt * 128:(tt + 1) * 128]
                yt = yout.tile([128, D], BF16, name=f"yt{rep}_{b}_{tt}",
                               tag="yt")
                for half in range(2):
                    py = wy_ps.tile([128, 512], F32,
                                    name=f"py{rep}_{b}_{tt}_{half}", tag="wy")
                    nc.tensor.matmul(py[:], lt,
                                     wp_s[:, half * 512:(half + 1) * 512],
                                     start=True, stop=True)
                    dst = yt[:, half * 512:(half + 1) * 512]
                    if act_copy_frac and (tt * 2 + half) % act_copy_frac == 0:
                        nc.scalar.activation(
                            dst, py[:], mybir.ActivationFunctionType.Copy)
                    else:
                        nc.vector.tensor_copy(dst, py[:])
                nc.sync.dma_start(
                    y_d.ap()[b * S + tt * 128: b * S + (tt + 1) * 128, :],
                    yt[:])

            NQT = QB // 128

            def make_attn_iter(rep, b, h, qb, ao):
                """Closures (sc, av, norm) for one (head, qblock) iteration,
                emitted piecewise by the flat software pipeline below.

                attn@V is flipped: stationary = pT [128 keys, 128 queries],
                moving = v [128 keys, VW]. Output poT [q, VW] carries the
                softmax denominator on the free dim and queries on
                partitions, so normalization is a plain per-partition
                tensor_scalar (no GpSimd broadcast); a PE transpose puts
                attn-out back in [ch, tok] layout for proj.
                """
                hs = slice(h * HD, (h + 1) * HD)
                qsl = slice(qb * QB, (qb + 1) * QB)
                st = {"pT": {}, "poT": None}

                def sc(g):
                    if g == 0:
                        # one poT tile (= one PSUM bank) per 128-query
                        # tile: each accumulation chain owns its bank, so
                        # its start=True cannot be scheduler-reordered
                        # against another chain's first write (the tile
                        # scheduler only preserves data deps, and matmul
                        # writes to different regions are independent).
                        st["poT"] = [
                            o_ps.tile([128, VW], F32,
                                      name=f"poT{rep}_{b}_{h}_{qb}_{qt}",
                                      tag="po")
                            for qt in range(NQT)]
                    ss = s_ps.tile([128, KG, QB], F32,
                                   name=f"ss{rep}_{b}_{h}_{qb}_{g}", tag="ss")
                    for j in range(KG):
                        kt = g * KG + j
                        # start=True on EVERY write: each j region is
                        # written exactly once, and a bank-wide pending
                        # mark over sibling regions is harmless to reads
                        # (pending only alters matmul-write behavior).
                        # Pairing starts (j%2) would instead depend on
                        # cross-j program order the scheduler won't keep.
                        nc.tensor.matmul(ss[:, j, :], kTp_v[b][:, kt, h, :],
                                         qT_b[b][:, qsl],
                                         start=True, stop=True)
                    pTg = p_pool.tile([128, KG, QB], BF16,
                                      name=f"pT{rep}_{b}_{h}_{qb}_{g}",
                                      tag="pT")
                    nc.scalar.activation(pTg[:], ss[:],
                                         mybir.ActivationFunctionType.Exp,
                                         scale=SCALE)
                    st["pT"][g] = pTg

                def av(g):
                    pTg = st["pT"].pop(g)
                    for j in range(KG):
                        kt = g * KG + j
                        vb = kt * HPC * VW + h * VW
                        for qt in range(NQT):
                            nc.tensor.matmul(
                                st["poT"][qt][:],
                                pTg[:, j, qt * 128:(qt + 1) * 128],
                                v_b[b][:, vb:vb + VW],
                                start=(kt == 0),
                                stop=(kt == NKT - 1))

                def norm():
                    for qt in range(NQT):
                        poT = st["poT"][qt]
                        rc = rcp_pool.tile([128, 1], F32,
                                           name=f"rc{rep}_{b}_{h}_{qb}_{qt}",
                                           tag="rc")
                        nc.vector.reciprocal(rc[:], poT[:, HD:VW])
                        aoT = aot_pool.tile(
                            [128, HD], BF16,
                            name=f"aoT{rep}_{b}_{h}_{qb}_{qt}", tag="aoT")
                        nc.vector.tensor_scalar_mul(aoT[:], poT[:, 0:HD],
                                                    rc[:])
                        # tr shares the po rotation (norm runs inline right
                        # after av(NG-1), so poT's readers are done before
                        # tr reuses its slot)
                        tr = tr_ps.tile([HD, 128], BF16,
                                        name=f"tr{rep}_{b}_{h}_{qb}_{qt}",
                                        tag="po")
                        nc.tensor.transpose(tr[:], aoT[:], ident[:])
                        ts0 = qb * QB + qt * 128
                        nc.vector.tensor_copy(ao[hs, ts0:ts0 + 128], tr[:])

                return sc, av, norm

            ao_tiles = {}
            for rep in range(reps):
                # phase B for batch 0: only the first 512-token block up
                # front; the remaining blocks stream between the score
                # groups of the first attention iteration (ACT starts exp
                # ~20us earlier)
                xt = load_x(rep, 0, split=True)
                if rep == 0:
                    setup()
                xt_next0 = load_x(rep, 1)
                # k first: iteration 0's sc(g) only needs kTp of block g,
                # so the k projection is the critical path; q of a block
                # is read no earlier than the iteration covering it, and
                # v(g) is consumed by av(g) one filler-slot later.
                emit_qkv_tb(rep, 0, xt, parts=("k", "q", "v"))
                if rep == 0:
                    # wp is first read by proj filler ~85us in; loading it
                    # here keeps it off the startup-critical DMA window
                    nc.sync.dma_start(wp_s[:], wp_d.ap()[:, :])
                b0_state = {"xt": xt_next0}

                def b0_filler(g):
                    tb = g + 1
                    if tb < TBB:
                        nxt = load_x(rep, tb + 1) if tb + 1 < TBB else None
                        emit_qkv_tb(rep, tb, b0_state["xt"],
                                    parts=("k", "v", "q"))
                        b0_state["xt"] = nxt

                # cross-batch x prefetch state for the qkv filler stream
                xt_state = {"xt": load_x(rep, TBB)}
                for b in range(B):
                    ao = ao_pool.tile([128, S], BF16, name=f"ao{rep}_{b}",
                                      tag="ao")
                    ao_tiles[(rep, b)] = ao
                    iters = []
                    for it in range(NIT):
                        h, qb = divmod(it, NQB)
                        iters.append(make_attn_iter(rep, b, h, qb, ao))

                    fillers_by_it = []
                    for it in range(NIT):
                        fillers = []
                        if 0 < b < B - 1:
                            tt = it
                            fillers.append(
                                lambda tt=tt: emit_proj_tt(
                                    rep, b - 1, ao_tiles[(rep, b - 1)], tt))
                        elif b == B - 1 and it < 8:
                            # the last batch has no qkv filler: front-load
                            # proj(b-1) at 2 blocks/iter in iters 0..7
                            for tt in (2 * it, 2 * it + 1):
                                fillers.append(
                                    lambda tt=tt: emit_proj_tt(
                                        rep, b - 1, ao_tiles[(rep, b - 1)],
                                        tt))
                        # one qkv part (q, k, or v: ~1.7us PE) per
                        # iteration for batch b+1; 12 parts over iterations
                        # 0..11 (shifted by one for b=0, whose iter 0 runs
                        # b0_filler)
                        pit = it - 1 if b == 0 else it
                        if b + 1 < B and 0 <= pit < 3 * TBB:
                            tbo, pi = divmod(pit, 3)
                            tb = (b + 1) * TBB + tbo
                            part = ("q", "k", "v")[pi]

                            def qkv_part(tb=tb, part=part):
                                xt = xt_state["xt"]
                                emit_qkv_tb(rep, tb, xt, parts=(part,))
                                if part == "q":
                                    xt_state["next"] = (
                                        load_x(rep, tb + 1)
                                        if tb + 1 < B * TBB else None)
                                elif part == "v":
                                    xt_state["xt"] = xt_state["next"]
                            fillers.append(qkv_part)
                        # last batch: its own proj blocks once their tokens
                        # are normalized (block tt ready after iter 8+tt//2)
                        if b == B - 1 and it >= 9:
                            for tt in (2 * (it - 9), 2 * (it - 9) + 1):
                                fillers.append(
                                    lambda tt=tt: emit_proj_tt(
                                        rep, b, ao, tt))
                        fillers_by_it.append(fillers)

                    def run_filler(it, g):
                        fl = fillers_by_it[it]
                        if g < NG - 1:
                            if g < len(fl):
                                fl[g]()
                        else:
                            for f in fl[NG - 1:]:
                                f()

                    # per-iteration emission: sc runs one group ahead of
                    # av; filler after sc (feeds the ACT pacer asap, then
                    # covers the rest of exp(g)'s latency before av(g)).
                    # b0's iteration 0 is special: its filler emits the
                    # qkv block that sc(g+1) reads, so filler goes first.
                    for it in range(NIT):
                        sc, av, norm = iters[it]
                        filler_first = b == 0 and it == 0
                        sc(0)
                        for g in range(NG):
                            if filler_first:
                                b0_filler(g)
                            if g + 1 < NG:
                                sc(g + 1)
                            if not filler_first:
                                run_filler(it, g)
                            av(g)
                        norm()
                # tail: the last two proj blocks of the last batch
                for tt in (S // 128 - 2, S // 128 - 1):
                    emit_proj_tt(rep, B - 1, ao_tiles[(rep, B - 1)], tt)

    nc.compile()
    return nc


def shard_inputs(x, w_qkv, b_qkv, w_proj, B=4, S=2048, D=1024, H=16):
    """Host-side sharding: returns in_maps for the 8 cores."""
    HD = D // H
    HPC = H // N_CORES
    CPC = HPC * HD
    T = B * S
    x = np.asarray(x, dtype=np.float32)
    w_qkv = np.asarray(w_qkv, dtype=np.float32)
    b_qkv = np.asarray(b_qkv, dtype=np.float32)
    w_proj = np.asarray(w_proj, dtype=np.float32)
    bf = ml_dtypes.bfloat16
    xT = np.ascontiguousarray(x.reshape(T, D).T.astype(bf))
    in_maps = []
    for c in range(N_CORES):
        sl = slice(c * CPC, (c + 1) * CPC)
        in_maps.append({
            "xT": xT,
            "wq": np.ascontiguousarray(w_qkv[:, 0 * D:1 * D][:, sl]
                                       .astype(bf)),
            "wk": np.ascontiguousarray(w_qkv[:, 1 * D:2 * D][:, sl]
                                       .astype(bf)),
            "wv": np.ascontiguousarray(w_qkv[:, 2 * D:3 * D][:, sl]
                                       .astype(bf)),
            "bq": np.ascontiguousarray(b_qkv[0 * D:1 * D][sl]).reshape(CPC, 1),
            "bk": np.ascontiguousarray(b_qkv[1 * D:2 * D][sl]).reshape(CPC, 1),
            "bv": np.ascontiguousarray(b_qkv[2 * D:3 * D][sl]).reshape(CPC, 1),
            "wp": np.ascontiguousarray(w_proj[sl, :].astype(bf)),
        })
    return in_maps


_NC_CACHE = {}


def _get_nc():
    if "nc" not in _NC_CACHE:
        _NC_CACHE["nc"] = build_core_program()
    return _NC_CACHE["nc"]


def kernel(x, w_qkv, b_qkv, w_proj, b_proj, _trace=False):
    B, S, D = 4, 2048, 1024
    nc = _get_nc()
    in_maps = shard_inputs(x, w_qkv, b_qkv, w_proj, B=B, S=S, D=D)
    res = run_bass_kernel_spmd(nc, in_maps, core_ids=list(range(N_CORES)),
                               trace=_trace)
    y = res.results[0]["y"].astype(np.float64)
    for i in range(1, N_CORES):
        y += res.results[i]["y"]
    y += np.asarray(b_proj, dtype=np.float64)
    out = y.astype(np.float32).reshape(B, S, D)
    if _trace:
        return out, res
    return out



# revision 64
# speedup vs baseline: 1.0344x; 1.0344x over previous
"""Multi-head attention (B=4, S=2048, D=1024, H=16) on 8 trn2 NeuronCores.

Sharding: tensor-parallel over heads - 2 heads per core. Each core computes
qkv for its 128 channels (column-parallel), full attention for its 8
(batch, head) pairs, and a row-parallel slice of the output projection.
Host gathers the 8 partial projection outputs (bf16) and sums them in
float64 (+ b_proj).

All matmul operands are bf16 (PSUM accumulation stays f32). Matmul time
is (moving columns) * ~0.42 ns, so the layout minimizes streamed columns:

  - Scores use a ZERO-PADDED kT: for each 128-key tile and head h, a
    [128,128] stationary block holds kT in partition rows 64h..64h+63 and
    zeros elsewhere; the moving operand is the stacked qT (head0 channels
    in partitions 0-63, head1 in 64-127). The zero rows kill the other
    head's contribution, and the output's full 128-key width is used, so
    scores hit the output-bound floor of S^2/128 cycles per (b,h).
  - attn@V is FLIPPED: stationary = pT [128 keys, 128 queries] (a slice
    of the exp output), moving = v [128 keys, 65]. That streams 65
    columns per (key tile x query tile) instead of 128+ per key tile —
    LDWEIGHTS of the next stationary hides behind the moving stream.
    v carries a ones column, so each accumulator poT [128 q, 65] ends
    with the softmax denominator at column 64: normalization is a plain
    per-partition DVE reciprocal + tensor_scalar multiply (no GpSimd),
    then a PE transpose puts attn-out back into [ch, tok] layout for the
    row-parallel projection.

Scheduling: softmax exp makes the ScalarE the steady-state pacer (~1.04us
per [128,1024] chunk), so each (head, qblock) iteration runs scores one
group ahead of attn@V, with filler work (output projection of batch b-1,
one 512-token qkv part of batch b+1 per iteration, and for the last batch
its own ready projection blocks) emitted between sc(g+1) and av(g).
Batch 0's qkv streams inside its first attention iteration (group g's
scores only need key block g).

PSUM discipline (the tile scheduler only preserves data deps, and
pending-zero marks are 2KB-bank granular): every non-accumulating matmul
write uses start=True, and every accumulation chain owns a whole bank
(one poT tile per 128-query tile), so no chain's start can be reordered
against another chain's first write.
"""

import numpy as np
import ml_dtypes

import concourse.bass as bass
import concourse.mybir as mybir
import concourse.tile as tile
from concourse import bacc
from concourse.bass_utils import run_bass_kernel_spmd
from concourse.masks import make_identity

F32 = mybir.dt.float32
BF16 = mybir.dt.bfloat16

N_CORES = 8


def build_core_program(B=4, S=2048, D=1024, H=16, QB=256, TB=512, reps=1,
                       bufs_x=5, bufs_pT=4, bufs_s=2, bufs_o=2, bufs_wy=2,
                       KG=4, act_copy_frac=0):
    """One core's program (SPMD: every core runs this on its own shard).

    act_copy_frac: every act_copy_frac-th projection psum->sbuf copy goes
    to ScalarE instead of DVE (0 = all on DVE).
    """
    HD = D // H                 # 64
    HPC = H // N_CORES          # heads per core = 2
    CPC = HPC * HD              # channels per core = 128
    T = B * S                   # tokens = 8192
    SCALE = float(HD) ** -0.5
    KT = 128                    # ki tile
    NKT = S // KT               # ki tiles per batch = 16
    NTT = T // KT               # token tiles total = 64
    VW = HD + 1                 # v tile width per head incl. ones col = 65
    NQB = S // QB               # q blocks per batch = 8
    KD = D // 128               # contraction tiles for qkv = 8
    KH = KD // 2
    NG = NKT // KG              # score groups per q block = 4
    TPB = TB // 128             # 128-token tiles per qkv block = 4
    TBB = S // TB               # qkv token blocks per batch = 4
    NIT = HPC * NQB             # attention iterations per batch = 16

    nc = bacc.Bacc("TRN2", target_bir_lowering=False, debug=False,
                   num_devices=N_CORES)

    xT_d = nc.dram_tensor("xT", [D, T], BF16, kind="ExternalInput")
    wq_d = nc.dram_tensor("wq", [D, CPC], BF16, kind="ExternalInput")
    wk_d = nc.dram_tensor("wk", [D, CPC], BF16, kind="ExternalInput")
    wv_d = nc.dram_tensor("wv", [D, CPC], BF16, kind="ExternalInput")
    bq_d = nc.dram_tensor("bq", [CPC, 1], F32, kind="ExternalInput")
    bk_d = nc.dram_tensor("bk", [CPC, 1], F32, kind="ExternalInput")
    bv_d = nc.dram_tensor("bv", [CPC, 1], F32, kind="ExternalInput")
    wp_d = nc.dram_tensor("wp", [CPC, D], BF16, kind="ExternalInput")
    y_d = nc.dram_tensor("y", [T, D], BF16, kind="ExternalOutput")

    with tile.TileContext(nc) as tc:
        with tc.tile_pool(name="const", bufs=1) as const, \
             tc.tile_pool(name="persist", bufs=1) as persist, \
             tc.tile_pool(name="xin", bufs=bufs_x) as xin, \
             tc.tile_pool(name="vtmp", bufs=3) as vtmp, \
             tc.tile_pool(name="pT", bufs=bufs_pT) as p_pool, \
             tc.tile_pool(name="ao", bufs=3) as ao_pool, \
             tc.tile_pool(name="rcp", bufs=3) as rcp_pool, \
             tc.tile_pool(name="aot", bufs=4) as aot_pool, \
             tc.tile_pool(name="yout", bufs=4) as yout, \
             tc.tile_pool(name="s_ps", bufs=bufs_s, space="PSUM") as s_ps, \
             tc.tile_pool(name="o_ps", bufs=bufs_o, space="PSUM") as o_ps, \
             tc.tile_pool(name="wy_ps", bufs=bufs_wy, space="PSUM") as wy_ps:
            tr_ps = o_ps  # poT and tr tiles share the o_ps bank rotation
            xT_r = xT_d.ap().rearrange("(a p) t -> p a t", p=128)
            # per-batch persistent tiles: whole-tile WAR tracking would
            # otherwise serialize each rep's first qkv writes behind the
            # previous rep's LAST attention reads (and filler qkv writes
            # behind unrelated batches' reads), blocking cross-rep overlap
            qT_b = [persist.tile([128, S], BF16, name=f"qT_{b}")
                    for b in range(B)]             # stacked head dims
            kTp_b = [persist.tile([128, 2 * S], BF16, name=f"kTp_{b}")
                     for b in range(B)]            # zero-padded per head
            v_b = [persist.tile([128, NKT * HPC * VW], BF16, name=f"v_{b}")
                   for b in range(B)]
            # padded-kT views: [128, key-tile, head, 128]
            kTp_v = [t[:].rearrange("p (t h c) -> p t h c", h=HPC, c=KT)
                     for t in kTp_b]

            ident = const.tile([128, 128], BF16)
            ones_bf = const.tile([128, 2], BF16)
            wq_s = const.tile([128, KD, 128], BF16)
            wk_s = const.tile([128, KD, 128], BF16)
            wv_s = const.tile([128, KD, 128], BF16)
            wp_s = const.tile([128, D], BF16)
            bq_s = const.tile([CPC, 1], F32)
            bk_s = const.tile([CPC, 1], F32)
            bv_s = const.tile([CPC, 1], F32)

            def setup():
                # k's weights first: block 0's k projection gates the
                # first score matmul
                for w_d, w_s in ((wk_d, wk_s), (wq_d, wq_s), (wv_d, wv_s)):
                    nc.sync.dma_start(
                        w_s[:], w_d.ap().rearrange("(a p) m -> p a m", p=128))
                nc.sync.dma_start(bq_s[:], bq_d.ap()[:, :])
                nc.sync.dma_start(bk_s[:], bk_d.ap()[:, :])
                nc.sync.dma_start(bv_s[:], bv_d.ap()[:, :])
                make_identity(nc, ident[:])
                nc.vector.memset(ones_bf[:], 1.0)
                for bb in range(B):
                    nc.vector.memset(kTp_b[bb][:], 0.0)
                    ones_cols = v_b[bb][:].rearrange(
                        "p (t h w) -> p t h w", t=NKT, h=HPC)[:, :, :, HD:VW]
                    nc.vector.tensor_copy(
                        ones_cols,
                        ones_bf[:, 0:1].to_broadcast([128, NKT, HPC, 1]))

            def load_x(rep, tb, split=False):
                xt = xin.tile([128, KD, TB], BF16, name=f"xt_{rep}_{tb}",
                              tag="xt")
                if split:
                    # per-2-kd DMAs so the first qkv matmul (kd=0) can
                    # start as soon as its slice lands
                    for k0 in range(0, KD, 2):
                        nc.sync.dma_start(
                            xt[:, k0:k0 + 2, :],
                            xT_r[:, k0:k0 + 2, tb * TB:(tb + 1) * TB])
                else:
                    nc.sync.dma_start(xt[:], xT_r[:, :, tb * TB:(tb + 1) * TB])
                return xt

            def emit_qkv_tb(rep, tb, xt, parts=("q", "k", "v")):
                """qkv projections + padded-kT / transposed-v writes for one
                512-token block."""
                bb, ltb = divmod(tb, TBB)   # batch, block-within-batch
                lsl = slice(ltb * TB, (ltb + 1) * TB)
                sel = {"q": (wq_s, bq_s), "k": (wk_s, bk_s),
                       "v": (wv_s, bv_s)}
                for which in parts:
                    w_s, b_s = sel[which]
                    ps = wy_ps.tile([128, TB], F32,
                                    name=f"ps_{which}{rep}_{tb}", tag="wy")
                    for kd in range(KD):
                        nc.tensor.matmul(ps[:], w_s[:, kd, :], xt[:, kd, :],
                                         start=(kd == 0), stop=(kd == KD - 1))
                    if which == "q":
                        nc.vector.tensor_scalar_add(qT_b[bb][:, lsl], ps[:],
                                                    b_s[:])
                    elif which == "k":
                        ps_r = ps[:].rearrange("p (t c) -> p t c", c=KT)
                        for h in range(HPC):
                            hs = slice(h * HD, (h + 1) * HD)
                            dst = kTp_v[bb][hs, ltb * TPB:(ltb + 1) * TPB,
                                           h, :]
                            nc.vector.tensor_scalar_add(
                                dst, ps_r[hs, :, :], b_s[hs, :])
                    else:
                        vt = vtmp.tile([128, TB], BF16, name=f"vt{rep}_{tb}",
                                       tag="vt")
                        nc.vector.tensor_scalar_add(vt[:], ps[:], b_s[:])
                        for j in range(TPB):
                            ti = ltb * TPB + j
                            pt = wy_ps.tile([128, 128], BF16,
                                            name=f"pt{rep}_{tb}_{j}",
                                            tag="wy")
                            nc.tensor.transpose(
                                pt[:], vt[:, j * 128:(j + 1) * 128], ident[:])
                            base = ti * HPC * VW
                            dst = v_b[bb][:, base:base + HPC * VW].rearrange(
                                "p (h w) -> p h w", h=HPC)[:, :, 0:HD]
                            src = pt[:].rearrange("p (h w) -> p h w", h=HPC)
                            nc.vector.tensor_copy(dst, src)

            def emit_proj_tt(rep, b, ao, tt):
                """output projection for one 128-token block of batch b."""
                lt = ao[:, tt * 128:(tt + 1) * 128]
                yt = yout.tile([128, D], BF16, name=f"yt{rep}_{b}_{tt}",
                               tag="yt")
                for half in range(2):
                    py = wy_ps.tile([128, 512], F32,
                                    name=f"py{rep}_{b}_{tt}_{half}", tag="wy")
                    nc.tensor.matmul(py[:], lt,
                                     wp_s[:, half * 512:(half + 1) * 512],
                                     start=True, stop=True)
                    dst = yt[:, half * 512:(half + 1) * 512]
                    if act_copy_frac and (tt * 2 + half) % act_copy_frac == 0:
                        nc.scalar.activation(
                            dst, py[:], mybir.ActivationFunctionType.Copy)
                    else:
                        nc.vector.tensor_copy(dst, py[:])
                nc.sync.dma_start(
                    y_d.ap()[b * S + tt * 128: b * S + (tt + 1) * 128, :],
                    yt[:])

            NQT = QB // 128

            def make_attn_iter(rep, b, h, qb, ao):
                """Closures (sc, av, norm) for one (head, qblock) iteration,
                emitted piecewise by the flat software pipeline below.

                attn@V is flipped: stationary = pT [128 keys, 128 queries],
                moving = v [128 keys, VW]. Output poT [q, VW] carries the
                softmax denominator on the free dim and queries on
                partitions, so normalization is a plain per-partition
                tensor_scalar (no GpSimd broadcast); a PE transpose puts
                attn-out back in [ch, tok] layout for proj.
                """
                hs = slice(h * HD, (h + 1) * HD)
                qsl = slice(qb * QB, (qb + 1) * QB)
                st = {"pT": {}, "poT": None}

                def sc(g):
                    if g == 0:
                        # one poT tile (= one PSUM bank) per 128-query
                        # tile: each accumulation chain owns its bank, so
                        # its start=True cannot be scheduler-reordered
                        # against another chain's first write (the tile
                        # scheduler only preserves data deps, and matmul
                        # writes to different regions are independent).
                        st["poT"] = [
                            o_ps.tile([128, VW], F32,
                                      name=f"poT{rep}_{b}_{h}_{qb}_{qt}",
                                      tag="po")
                            for qt in range(NQT)]
                    ss = s_ps.tile([128, KG, QB], F32,
                                   name=f"ss{rep}_{b}_{h}_{qb}_{g}", tag="ss")
                    for j in range(KG):
                        kt = g * KG + j
                        # start=True on EVERY write: each j region is
                        # written exactly once, and a bank-wide pending
                        # mark over sibling regions is harmless to reads
                        # (pending only alters matmul-write behavior).
                        # Pairing starts (j%2) would instead depend on
                        # cross-j program order the scheduler won't keep.
                        nc.tensor.matmul(ss[:, j, :], kTp_v[b][:, kt, h, :],
                                         qT_b[b][:, qsl],
                                         start=True, stop=True)
                    pTg = p_pool.tile([128, KG, QB], BF16,
                                      name=f"pT{rep}_{b}_{h}_{qb}_{g}",
                                      tag="pT")
                    nc.scalar.activation(pTg[:], ss[:],
                                         mybir.ActivationFunctionType.Exp,
                                         scale=SCALE)
                    st["pT"][g] = pTg

                def av(g):
                    pTg = st["pT"].pop(g)
                    for j in range(KG):
                        kt = g * KG + j
                        vb = kt * HPC * VW + h * VW
                        for qt in range(NQT):
                            nc.tensor.matmul(
                                st["poT"][qt][:],
                                pTg[:, j, qt * 128:(qt + 1) * 128],
                                v_b[b][:, vb:vb + VW],
                                start=(kt == 0),
                                stop=(kt == NKT - 1))

                def norm():
                    for qt in range(NQT):
                        poT = st["poT"][qt]
                        rc = rcp_pool.tile([128, 1], F32,
                                           name=f"rc{rep}_{b}_{h}_{qb}_{qt}",
                                           tag="rc")
                        nc.vector.reciprocal(rc[:], poT[:, HD:VW])
                        aoT = aot_pool.tile(
                            [128, HD], BF16,
                            name=f"aoT{rep}_{b}_{h}_{qb}_{qt}", tag="aoT")
                        nc.vector.tensor_scalar_mul(aoT[:], poT[:, 0:HD],
                                                    rc[:])
                        # tr shares the po rotation (norm runs inline right
                        # after av(NG-1), so poT's readers are done before
                        # tr reuses its slot)
                        tr = tr_ps.tile([HD, 128], BF16,
                                        name=f"tr{rep}_{b}_{h}_{qb}_{qt}",
                                        tag="po")
                        nc.tensor.transpose(tr[:], aoT[:], ident[:])
                        ts0 = qb * QB + qt * 128
                        nc.vector.tensor_copy(ao[hs, ts0:ts0 + 128], tr[:])

                return sc, av, norm

            ao_tiles = {}
            for rep in range(reps):
                # phase B for batch 0: only the first 512-token block up
                # front; the remaining blocks stream between the score
                # groups of the first attention iteration (ACT starts exp
                # ~20us earlier)
                xt = load_x(rep, 0, split=True)
                if rep == 0:
                    setup()
                xt_next0 = load_x(rep, 1)
                # k first: iteration 0's sc(g) only needs kTp of block g,
                # so the k projection is the critical path; q of a block
                # is read no earlier than the iteration covering it, and
                # v(g) is consumed by av(g) one filler-slot later.
                emit_qkv_tb(rep, 0, xt, parts=("k", "q", "v"))
                if rep == 0:
                    # wp is first read by proj filler ~85us in; loading it
                    # here keeps it off the startup-critical DMA window
                    nc.sync.dma_start(wp_s[:], wp_d.ap()[:, :])
                b0_state = {"xt": xt_next0}

                def b0_filler(g):
                    tb = g + 1
                    if tb < TBB:
                        nxt = load_x(rep, tb + 1) if tb + 1 < TBB else None
                        emit_qkv_tb(rep, tb, b0_state["xt"],
                                    parts=("k", "v", "q"))
                        b0_state["xt"] = nxt

                # cross-batch x prefetch state for the qkv filler stream
                xt_state = {"xt": load_x(rep, TBB)}
                for b in range(B):
                    ao = ao_pool.tile([128, S], BF16, name=f"ao{rep}_{b}",
                                      tag="ao")
                    ao_tiles[(rep, b)] = ao
                    iters = []
                    for it in range(NIT):
                        h, qb = divmod(it, NQB)
                        iters.append(make_attn_iter(rep, b, h, qb, ao))

                    fillers_by_it = []
                    for it in range(NIT):
                        fillers = []
                        if 0 < b < B - 1:
                            tt = it
                            fillers.append(
                                lambda tt=tt: emit_proj_tt(
                                    rep, b - 1, ao_tiles[(rep, b - 1)], tt))
                        elif b == B - 1 and it < 8:
                            # the last batch has no qkv filler: front-load
                            # proj(b-1) at 2 blocks/iter in iters 0..7
                            for tt in (2 * it, 2 * it + 1):
                                fillers.append(
                                    lambda tt=tt: emit_proj_tt(
                                        rep, b - 1, ao_tiles[(rep, b - 1)],
                                        tt))
                        # one qkv part (q, k, or v: ~1.7us PE) per
                        # iteration for batch b+1; 12 parts over iterations
                        # 0..11 (shifted by one for b=0, whose iter 0 runs
                        # b0_filler)
                        pit = it - 1 if b == 0 else it
                        if b + 1 < B and 0 <= pit < 3 * TBB:
                            tbo, pi = divmod(pit, 3)
                            tb = (b + 1) * TBB + tbo
                            part = ("q", "k", "v")[pi]

                            def qkv_part(tb=tb, part=part):
                                xt = xt_state["xt"]
                                emit_qkv_tb(rep, tb, xt, parts=(part,))
                                if part == "q":
                                    xt_state["next"] = (
                                        load_x(rep, tb + 1)
                                        if tb + 1 < B * TBB else None)
                                elif part == "v":
                                    xt_state["xt"] = xt_state["next"]
                            fillers.append(qkv_part)
                        # last batch: its own proj blocks once their tokens
                        # are normalized (block tt ready after iter 8+tt//2)
                        if b == B - 1 and it >= 9:
                            for tt in (2 * (it - 9), 2 * (it - 9) + 1):
                                fillers.append(
                                    lambda tt=tt: emit_proj_tt(
                                        rep, b, ao, tt))
                        fillers_by_it.append(fillers)

                    def run_filler(it, g):
                        fl = fillers_by_it[it]
                        if g < NG - 1:
                            if g < len(fl):
                                fl[g]()
                        else:
                            for f in fl[NG - 1:]:
                                f()

                    # per-iteration emission: sc runs one group ahead of
                    # av; filler after sc (feeds the ACT pacer asap, then
                    # covers the rest of exp(g)'s latency before av(g)).
                    # b0's iteration 0 is special: its filler emits the
                    # qkv block that sc(g+1) reads, so filler goes first.
                    for it in range(NIT):
                        sc, av, norm = iters[it]
                        filler_first = b == 0 and it == 0
                        sc(0)
                        for g in range(NG):
                            if filler_first:
                                b0_filler(g)
                            if g + 1 < NG:
                                sc(g + 1)
                            if not filler_first:
                                run_filler(it, g)
                            av(g)
                        norm()
                # tail: the last two proj blocks of the last batch
                for tt in (S // 128 - 2, S // 128 - 1):
                    emit_proj_tt(rep, B - 1, ao_tiles[(rep, B - 1)], tt)

    nc.compile()
    return nc


def shard_inputs(x, w_qkv, b_qkv, w_proj, B=4, S=2048, D=1024, H=16):
    """Host-side sharding: returns in_maps for the 8 cores."""
    HD = D // H
    HPC = H // N_CORES
    CPC = HPC * HD
    T = B * S
    x = np.asarray(x, dtype=np.float32)
    w_qkv = np.asarray(w_qkv, dtype=np.float32)
    b_qkv = np.asarray(b_qkv, dtype=np.float32)
    w_proj = np.asarray(w_proj, dtype=np.float32)
    bf = ml_dtypes.bfloat16
    xT = np.ascontiguousarray(x.reshape(T, D).T.astype(bf))
    in_maps = []
    for c in range(N_CORES):
        sl = slice(c * CPC, (c + 1) * CPC)
        in_maps.append({
            "xT": xT,
            "wq": np.ascontiguousarray(w_qkv[:, 0 * D:1 * D][:, sl]
                                       .astype(bf)),
            "wk": np.ascontiguousarray(w_qkv[:, 1 * D:2 * D][:, sl]
                                       .astype(bf)),
            "wv": np.ascontiguousarray(w_qkv[:, 2 * D:3 * D][:, sl]
                                       .astype(bf)),
            "bq": np.ascontiguousarray(b_qkv[0 * D:1 * D][sl]).reshape(CPC, 1),
            "bk": np.ascontiguousarray(b_qkv[1 * D:2 * D][sl]).reshape(CPC, 1),
            "bv": np.ascontiguousarray(b_qkv[2 * D:3 * D][sl]).reshape(CPC, 1),
            "wp": np.ascontiguousarray(w_proj[sl, :].astype(bf)),
        })
    return in_maps


_NC_CACHE = {}


def _get_nc():
    if "nc" not in _NC_CACHE:
        _NC_CACHE["nc"] = build_core_program()
    return _NC_CACHE["nc"]


def kernel(x, w_qkv, b_qkv, w_proj, b_proj, _trace=False):
    B, S, D = 4, 2048, 1024
    nc = _get_nc()
    in_maps = shard_inputs(x, w_qkv, b_qkv, w_proj, B=B, S=S, D=D)
    res = run_bass_kernel_spmd(nc, in_maps, core_ids=list(range(N_CORES)),
                               trace=_trace)
    y = res.results[0]["y"].astype(np.float64)
    for i in range(1, N_CORES):
        y += res.results[i]["y"]
    y += np.asarray(b_proj, dtype=np.float64)
    out = y.astype(np.float32).reshape(B, S, D)
    if _trace:
        return out, res
    return out

